# revision 17
# baseline (speedup 1.0000x reference)
"""Trainium2 Bass kernel for nn_AdaptiveFusion (segment_reduce).

Strategy: shard intersections by SEGMENT RANGE (host sorts rows by segment id
during the shard step). Each of the 8 cores owns a disjoint range of segments
and all rows belonging to them, so the segment reduction is fully local and no
collectives are needed. Rows are packed into 1024-row chunks aligned to segment
boundaries; each chunk owns a private 128-slot window of segment slots, making
the whole computation window-local: segment sums, the linear+sigmoid, and the
expand-multiply all happen per-window entirely in SBUF/PSUM in ONE fused pass
(feats are read exactly once in bf16; no DRAM scratch, no dynamic addressing).

Per window (128 slots, 1024 rows = 8 sub-tiles of 128):
  sums:   one-hot masks (rank == iota, DVE) -> 8 matmuls -> psum [128, 257]
          (256 feature sums + count column from the host-baked ones column)
  mid:    inv = 1/max(count,1); PE-transpose sums; (sumsT.T @ W.T) accumulated
          in psum; sigmoid with per-partition scale=inv -> win [128,256] bf16
  expand: host-baked transposed one-hot (fp8, exact) as matmul stationary
          -> 8 matmuls (maskT.T @ win) select each row's weight vector ->
          psum drain split ACT/DVE -> multiply with feats -> out bf16

Row r of big-chunk c lives at DRAM position 2048c + 16p + j (partition p,
sub-slot j) so every DMA moves 8KB contiguous per partition.
"""

import os
import numpy as np
import ml_dtypes

bf16 = ml_dtypes.bfloat16
fp8 = ml_dtypes.float8_e4m3

# ---- hardcoded problem geometry ----
N = 500000
S = 50000
D = 256
NCORES = 8

R = 1024           # rows per window-chunk
NCH = 64           # window-chunks per core
NCAP = R * NCH     # 65536 padded rows per core
TC = 257           # 256 feature sums + 1 count
T = R // 128       # sub-tiles per window (8)
BC = 2             # window-chunks per big DMA chunk (2048 rows)
NBC = NCH // BC    # 32 big chunks

LAST_EXEC_NS = None
LAST_RESULTS = None


def _build_graph(reps=1):
    from concourse import bacc, mybir
    import concourse.tile as tile
    from concourse.masks import make_identity

    f32 = mybir.dt.float32
    bf = mybir.dt.bfloat16
    f8 = mybir.dt.float8e4
    i32 = mybir.dt.int32

    nc = bacc.Bacc(None, target_bir_lowering=False)

    feats = nc.declare_dram_parameter("feats", [NCAP, TC], bf, isOutput=False)
    ur = nc.declare_dram_parameter("ur", [128, NCH * T], bf, isOutput=False)
    mskt_h = nc.declare_dram_parameter("mskt_h", [NBC, 128, BC * R], f8, isOutput=False)
    wt = nc.declare_dram_parameter("wt", [2, 128, 256], bf, isOutput=False)
    out = nc.declare_dram_parameter("out", [NCAP, 256], bf, isOutput=True)

    # row r = 2048*c + 16*p + j  ->  [c][p, j, :]  (8KB contiguous / partition)
    feats_r = feats[:].rearrange("(c p j) e -> c p j e", p=128, j=BC * T)
    out_r = out[:].rearrange("(c p j) e -> c p j e", p=128, j=BC * T)

    with tile.TileContext(nc) as tc:
        with (
            tc.tile_pool(name="const", bufs=1) as constp,
            tc.tile_pool(name="sb", bufs=4) as sb,
            tc.tile_pool(name="stg", bufs=2) as stgp,
            tc.tile_pool(name="ps", bufs=2, space="PSUM") as psp,
            tc.tile_pool(name="pst", bufs=1, space="PSUM") as pstp,
            tc.tile_pool(name="psw", bufs=1, space="PSUM") as pswp,
            tc.tile_pool(name="ex", bufs=3, space="PSUM") as exp_,
        ):
            # ---- constants ----
            iota_i = constp.tile([128, T, 128], i32)
            nc.gpsimd.iota(iota_i[:], pattern=[[0, T], [1, 128]], base=0,
                           channel_multiplier=0)
            iota_rb = constp.tile([128, T, 128], bf)  # value = free index m
            nc.vector.tensor_copy(iota_rb[:], iota_i[:])
            ident = constp.tile([128, 128], bf)
            make_identity(nc, ident[:])
            wt_sb = constp.tile([128, 2, 256], bf)
            nc.sync.dma_start(wt_sb[:], wt[:].rearrange("h k n -> k h n"))
            ur_sb = constp.tile([128, NCH * T], bf)
            nc.sync.dma_start(ur_sb[:], ur[:])

            for c in range(reps * NBC):
                c = c % NBC
                mov = sb.tile([128, BC * T, TC], bf, tag="mov")
                nc.sync.dma_start(mov[:], feats_r[c])
                mskt = sb.tile([128, BC * R], f8, tag="mskt")
                nc.sync.dma_start(mskt[:], mskt_h[:][c])
                ot = stgp.tile([128, BC * T, 256], bf, tag="ot")
                for w in range(BC):
                    wc = BC * c + w          # global window index
                    # -- segment sums + counts --
                    msk = sb.tile([128, T, 128], bf, tag="msk")
                    nc.vector.tensor_tensor(
                        out=msk[:],
                        in0=ur_sb[:, wc * T:(wc + 1) * T][:, :, None]
                            .to_broadcast([128, T, 128]),
                        in1=iota_rb[:],
                        op=mybir.AluOpType.is_equal,
                    )
                    ps = psp.tile([128, TC], f32, tag="ps")
                    for t in range(T):
                        nc.tensor.matmul(
                            ps[:], lhsT=msk[:, t, :], rhs=mov[:, T * w + t, :],
                            start=(t == 0), stop=(t == T - 1),
                        )
                    # -- weights: sigmoid((sums @ W.T) / count) --
                    cnt = sb.tile([128, 1], f32, tag="cnt")
                    nc.vector.tensor_scalar_max(cnt[:], ps[:, 256:257], 1.0)
                    inv = sb.tile([128, 1], f32, tag="inv")
                    nc.vector.reciprocal(inv[:], cnt[:])
                    sums = sb.tile([128, 256], bf, tag="sums")
                    nc.scalar.activation(sums[:], ps[:, 0:256],
                                         mybir.ActivationFunctionType.Copy)
                    pst = pstp.tile([128, 2, 128], bf, tag="pst")
                    for h in range(2):
                        nc.tensor.transpose(pst[:, h, :],
                                            sums[:, 128 * h:128 * (h + 1)], ident[:])
                    at = sb.tile([128, 2, 128], bf, tag="at")
                    nc.vector.tensor_copy(at[:], pst[:])
                    psw = pswp.tile([128, 256], f32, tag="psw")
                    for h in range(2):
                        nc.tensor.matmul(
                            psw[:], lhsT=at[:, h, :], rhs=wt_sb[:, h, :],
                            start=(h == 0), stop=(h == 1),
                        )
                    win = sb.tile([128, 256], bf, tag="win")
                    nc.scalar.activation(win[:], psw[:],
                                         mybir.ActivationFunctionType.Sigmoid,
                                         scale=inv[:])
                    # -- expand weights back to rows and multiply --
                    for half in range(T // 2):
                        ex = exp_.tile([128, 2, 256], f32, tag="ex")
                        for i in range(2):
                            t = 2 * half + i
                            nc.tensor.matmul(ex[:, i, :],
                                             lhsT=mskt[:, w * R + 128 * t:
                                                       w * R + 128 * (t + 1)],
                                             rhs=win[:], start=True, stop=True)
                        j = T * w + 2 * half
                        if half % 2 == 0:
                            exb = sb.tile([128, 2, 256], bf, tag="exb")
                            nc.scalar.activation(exb[:], ex[:],
                                                 mybir.ActivationFunctionType.Copy)
                            nc.vector.tensor_tensor(
                                out=ot[:, j:j + 2, :], in0=ft_slice(mov, j),
                                in1=exb[:], op=mybir.AluOpType.mult,
                            )
                        else:
                            nc.vector.tensor_tensor(
                                out=ot[:, j:j + 2, :], in0=ft_slice(mov, j),
                                in1=ex[:], op=mybir.AluOpType.mult,
                            )
                nc.sync.dma_start(out_r[c], ot[:])

    nc.compile()
    return nc


def ft_slice(mov, j):
    # feats columns 0:256 of sub-tiles j, j+1 as [128, 2, 256]
    return mov[:, j:j + 2, 0:256]


def _prepare_shards(feats_f32, idx):
    """Sort rows by segment, cut into 8 segment-range core shards, pack each
    into 512-row segment-aligned chunks with private 128-slot windows."""
    n = idx.shape[0]
    order = np.argsort(idx, kind="stable")
    sidx = idx[order].astype(np.int64)

    cuts = [0]
    for c in range(1, NCORES):
        target = c * n // NCORES
        seg = sidx[target]
        cuts.append(int(np.searchsorted(sidx, seg, "left")))
    cuts.append(n)

    feats_list, ur_list, urt_list, rowsrc_list = [], [], [], []

    for c in range(NCORES):
        lo, hi = cuts[c], cuts[c + 1]

        chunk_starts, chunk_rows, chunk_spans = [], [], []
        pos = lo
        while pos < hi:
            end = min(pos + R, hi)
            if end < hi:
                segstart = int(np.searchsorted(sidx, sidx[end], "left"))
                if segstart > pos:
                    end = segstart
            nsegs = len(np.unique(sidx[pos:end]))
            while nsegs > 126:
                u = np.unique(sidx[pos:end])
                end = int(np.searchsorted(sidx, u[126], "left"))
                nsegs = 126
            chunk_starts.append(pos)
            chunk_rows.append(end - pos)
            chunk_spans.append(nsegs)
            pos = end
        assert len(chunk_starts) <= NCH, f"core {c}: {len(chunk_starts)} chunks > {NCH}"

        fz = np.zeros((NCAP, TC), dtype=bf16)
        ranks_all = np.zeros((NCH, R), dtype=np.int64)
        rs = np.full((NCAP,), -1, dtype=np.int64)

        for k in range(len(chunk_starts)):
            p0, nr, span = chunk_starts[k], chunk_rows[k], chunk_spans[k]
            rows = order[p0:p0 + nr]
            segs = sidx[p0:p0 + nr]
            rank = np.zeros(nr, dtype=np.int64)
            rank[1:] = np.cumsum(segs[1:] != segs[:-1])
            base = k * R
            fz[base:base + nr, :256] = feats_f32[rows].astype(bf16)
            fz[base:base + R, 256] = 1.0
            rs[base:base + nr] = rows
            ranks_full = np.full(R, span, dtype=np.int64)  # pad rows -> pad slot
            ranks_full[:nr] = rank
            ranks_all[k] = ranks_full

        urz = ranks_all.reshape(NCH, T, 128).transpose(2, 0, 1).reshape(128, NCH * T)
        oh = (ranks_all[:, None, :] == np.arange(128)[None, :, None])
        urtz = oh.reshape(NBC, BC, 128, R).transpose(0, 2, 1, 3).reshape(NBC, 128, BC * R)

        # permute chunk-linear rows into the device block layout:
        # chunk k, sorted index i -> 2048*(k//BC) + (BC*T)*p + T*(k%BC) + t
        # with p = i % 128, t = i // 128
        kk = np.arange(NCH)[:, None]
        ii = np.arange(R)[None, :]
        pos = (R * BC) * (kk // BC) + (BC * T) * (ii % 128) + T * (kk % BC) + ii // 128
        pos_flat = pos.ravel()
        fz_b = np.zeros_like(fz)
        fz_b[pos_flat] = fz
        rs_b = np.full_like(rs, -1)
        rs_b[pos_flat] = rs
        fz, rs = fz_b, rs_b

        feats_list.append(fz)
        ur_list.append(np.ascontiguousarray(urz).astype(bf16))
        urt_list.append(np.ascontiguousarray(urtz).astype(fp8))
        rowsrc_list.append(rs)

    return feats_list, ur_list, urt_list, rowsrc_list


def kernel(intersect_rgb_feat, intersect_voxel_feat, miss_ray_intersect_idx,
           total_miss_sample_num, W):
    global LAST_EXEC_NS, LAST_RESULTS
    from concourse.bass_utils import run_bass_kernel_spmd

    rgb = np.asarray(intersect_rgb_feat, dtype=np.float32)
    vox = np.asarray(intersect_voxel_feat, dtype=np.float32)
    idx = np.asarray(miss_ray_intersect_idx).astype(np.int64)
    Wm = np.asarray(W, dtype=np.float32)
    assert rgb.shape == (N, 128) and vox.shape == (N, 128)
    assert int(total_miss_sample_num) == S

    feats_f32 = np.concatenate([rgb, vox], axis=1)
    feats_list, ur_list, urt_list, rowsrc_list = _prepare_shards(feats_f32, idx)

    wt_host = np.ascontiguousarray(Wm.T.reshape(2, 128, 256)).astype(bf16)

    nc = _build_graph()

    in_maps = []
    for c in range(NCORES):
        in_maps.append({
            "feats": feats_list[c],
            "ur": ur_list[c],
            "mskt_h": urt_list[c],
            "wt": wt_host,
        })

    trace = bool(os.environ.get("BASS_TRACE"))
    res = run_bass_kernel_spmd(nc, in_maps, core_ids=list(range(NCORES)),
                               trace=trace)
    LAST_EXEC_NS = res.exec_time_ns
    LAST_RESULTS = res

    out_full = np.zeros((N, D), dtype=np.float32)
    for c in range(NCORES):
        o = np.asarray(res.results[c]["out"]).astype(np.float32)
        rs = rowsrc_list[c]
        valid = rs >= 0
        out_full[rs[valid]] = o[valid]
    return out_full



# revision 18
# speedup vs baseline: 1.0334x; 1.0334x over previous
"""Trainium2 Bass kernel for nn_AdaptiveFusion (segment_reduce).

Strategy: shard intersections by SEGMENT RANGE (host sorts rows by segment id
during the shard step). Each of the 8 cores owns a disjoint range of segments
and all rows belonging to them, so the segment reduction is fully local and no
collectives are needed. Rows are packed into 1024-row chunks aligned to segment
boundaries; each chunk owns a private 128-slot window of segment slots, making
the whole computation window-local: segment sums, the linear+sigmoid, and the
expand-multiply all happen per-window entirely in SBUF/PSUM in ONE fused pass
(feats are read exactly once in bf16; no DRAM scratch, no dynamic addressing).

Per window (128 slots, 1024 rows = 8 sub-tiles of 128):
  sums:   one-hot masks (rank == iota, DVE) -> 8 matmuls -> psum [128, 257]
          (256 feature sums + count column from the host-baked ones column)
  mid:    inv = 1/max(count,1); PE-transpose sums; (sumsT.T @ W.T) accumulated
          in psum; sigmoid with per-partition scale=inv -> win [128,256] bf16
  expand: host-baked transposed one-hot (fp8, exact) as matmul stationary
          -> 8 matmuls (maskT.T @ win) select each row's weight vector ->
          psum drain split ACT/DVE -> multiply with feats -> out bf16

Row r of big-chunk c lives at DRAM position 2048c + 16p + j (partition p,
sub-slot j) so every DMA moves 8KB contiguous per partition.
"""

import os
import numpy as np
import ml_dtypes

bf16 = ml_dtypes.bfloat16
fp8 = ml_dtypes.float8_e4m3

# ---- hardcoded problem geometry ----
N = 500000
S = 50000
D = 256
NCORES = 8

R = 1024           # rows per window-chunk
NCH = 64           # window-chunks per core
NCAP = R * NCH     # 65536 padded rows per core
TC = 257           # 256 feature sums + 1 count
T = R // 128       # sub-tiles per window (8)
BC = 2             # window-chunks per big DMA chunk (2048 rows)
NBC = NCH // BC    # 32 big chunks

LAST_EXEC_NS = None
LAST_RESULTS = None


def _build_graph(reps=1):
    from concourse import bacc, mybir
    import concourse.tile as tile
    from concourse.masks import make_identity

    f32 = mybir.dt.float32
    bf = mybir.dt.bfloat16
    f8 = mybir.dt.float8e4
    i32 = mybir.dt.int32

    nc = bacc.Bacc(None, target_bir_lowering=False)

    feats = nc.declare_dram_parameter("feats", [NCAP, TC], bf, isOutput=False)
    ur = nc.declare_dram_parameter("ur", [128, NCH * T], bf, isOutput=False)
    mskt_h = nc.declare_dram_parameter("mskt_h", [NBC, 128, BC * R], f8, isOutput=False)
    wt = nc.declare_dram_parameter("wt", [2, 128, 256], bf, isOutput=False)
    out = nc.declare_dram_parameter("out", [NCAP, 256], bf, isOutput=True)

    # row r = 2048*c + 16*p + j  ->  [c][p, j, :]  (8KB contiguous / partition)
    feats_r = feats[:].rearrange("(c p j) e -> c p j e", p=128, j=BC * T)
    out_r = out[:].rearrange("(c p j) e -> c p j e", p=128, j=BC * T)

    with tile.TileContext(nc) as tc:
        with (
            tc.tile_pool(name="const", bufs=1) as constp,
            tc.tile_pool(name="sb", bufs=5) as sb,
            tc.tile_pool(name="stg", bufs=2) as stgp,
            tc.tile_pool(name="ps", bufs=2, space="PSUM") as psp,
            tc.tile_pool(name="pst", bufs=1, space="PSUM") as pstp,
            tc.tile_pool(name="psw", bufs=1, space="PSUM") as pswp,
            tc.tile_pool(name="ex", bufs=4, space="PSUM") as exp_,
        ):
            # ---- constants ----
            iota_i = constp.tile([128, T, 128], i32)
            nc.gpsimd.iota(iota_i[:], pattern=[[0, T], [1, 128]], base=0,
                           channel_multiplier=0)
            iota_rb = constp.tile([128, T, 128], bf)  # value = free index m
            nc.vector.tensor_copy(iota_rb[:], iota_i[:])
            ident = constp.tile([128, 128], bf)
            make_identity(nc, ident[:])
            wt_sb = constp.tile([128, 2, 256], bf)
            nc.sync.dma_start(wt_sb[:], wt[:].rearrange("h k n -> k h n"))
            ur_sb = constp.tile([128, NCH * T], bf)
            nc.sync.dma_start(ur_sb[:], ur[:])

            for c in range(reps * NBC):
                c = c % NBC
                mov = sb.tile([128, BC * T, TC], bf, tag="mov")
                nc.sync.dma_start(mov[:], feats_r[c])
                mskt = sb.tile([128, BC * R], f8, tag="mskt")
                nc.sync.dma_start(mskt[:], mskt_h[:][c])
                ot = stgp.tile([128, BC * T, 256], bf, tag="ot")
                for w in range(BC):
                    wc = BC * c + w          # global window index
                    # -- segment sums + counts --
                    msk = sb.tile([128, T, 128], bf, tag="msk")
                    nc.vector.tensor_tensor(
                        out=msk[:],
                        in0=ur_sb[:, wc * T:(wc + 1) * T][:, :, None]
                            .to_broadcast([128, T, 128]),
                        in1=iota_rb[:],
                        op=mybir.AluOpType.is_equal,
                    )
                    ps = psp.tile([128, TC], f32, tag="ps")
                    for t in range(T):
                        nc.tensor.matmul(
                            ps[:], lhsT=msk[:, t, :], rhs=mov[:, T * w + t, :],
                            start=(t == 0), stop=(t == T - 1),
                        )
                    # -- weights: sigmoid((sums @ W.T) / count) --
                    cnt = sb.tile([128, 1], f32, tag="cnt")
                    nc.vector.tensor_scalar_max(cnt[:], ps[:, 256:257], 1.0)
                    inv = sb.tile([128, 1], f32, tag="inv")
                    nc.vector.reciprocal(inv[:], cnt[:])
                    sums = sb.tile([128, 256], bf, tag="sums")
                    nc.scalar.activation(sums[:], ps[:, 0:256],
                                         mybir.ActivationFunctionType.Copy)
                    pst = pstp.tile([128, 2, 128], bf, tag="pst")
                    for h in range(2):
                        nc.tensor.transpose(pst[:, h, :],
                                            sums[:, 128 * h:128 * (h + 1)], ident[:])
                    at = sb.tile([128, 2, 128], bf, tag="at")
                    nc.vector.tensor_copy(at[:], pst[:])
                    psw = pswp.tile([128, 256], f32, tag="psw")
                    for h in range(2):
                        nc.tensor.matmul(
                            psw[:], lhsT=at[:, h, :], rhs=wt_sb[:, h, :],
                            start=(h == 0), stop=(h == 1),
                        )
                    win = sb.tile([128, 256], bf, tag="win")
                    nc.scalar.activation(win[:], psw[:],
                                         mybir.ActivationFunctionType.Sigmoid,
                                         scale=inv[:])
                    # -- expand weights back to rows and multiply --
                    for half in range(T // 2):
                        ex = exp_.tile([128, 2, 256], f32, tag="ex")
                        for i in range(2):
                            t = 2 * half + i
                            nc.tensor.matmul(ex[:, i, :],
                                             lhsT=mskt[:, w * R + 128 * t:
                                                       w * R + 128 * (t + 1)],
                                             rhs=win[:], start=True, stop=True)
                        j = T * w + 2 * half
                        if half == 0:
                            exb = sb.tile([128, 2, 256], bf, tag="exb")
                            nc.scalar.activation(exb[:], ex[:],
                                                 mybir.ActivationFunctionType.Copy)
                            nc.gpsimd.tensor_tensor(
                                out=ot[:, j:j + 2, :], in0=ft_slice(mov, j),
                                in1=exb[:], op=mybir.AluOpType.mult,
                            )
                        else:
                            nc.vector.tensor_tensor(
                                out=ot[:, j:j + 2, :], in0=ft_slice(mov, j),
                                in1=ex[:], op=mybir.AluOpType.mult,
                            )
                nc.sync.dma_start(out_r[c], ot[:])

    nc.compile()
    return nc


def ft_slice(mov, j):
    # feats columns 0:256 of sub-tiles j, j+1 as [128, 2, 256]
    return mov[:, j:j + 2, 0:256]


def _prepare_shards(feats_f32, idx):
    """Sort rows by segment, cut into 8 segment-range core shards, pack each
    into 512-row segment-aligned chunks with private 128-slot windows."""
    n = idx.shape[0]
    order = np.argsort(idx, kind="stable")
    sidx = idx[order].astype(np.int64)

    cuts = [0]
    for c in range(1, NCORES):
        target = c * n // NCORES
        seg = sidx[target]
        cuts.append(int(np.searchsorted(sidx, seg, "left")))
    cuts.append(n)

    feats_list, ur_list, urt_list, rowsrc_list = [], [], [], []

    for c in range(NCORES):
        lo, hi = cuts[c], cuts[c + 1]

        chunk_starts, chunk_rows, chunk_spans = [], [], []
        pos = lo
        while pos < hi:
            end = min(pos + R, hi)
            if end < hi:
                segstart = int(np.searchsorted(sidx, sidx[end], "left"))
                if segstart > pos:
                    end = segstart
            nsegs = len(np.unique(sidx[pos:end]))
            while nsegs > 126:
                u = np.unique(sidx[pos:end])
                end = int(np.searchsorted(sidx, u[126], "left"))
                nsegs = 126
            chunk_starts.append(pos)
            chunk_rows.append(end - pos)
            chunk_spans.append(nsegs)
            pos = end
        assert len(chunk_starts) <= NCH, f"core {c}: {len(chunk_starts)} chunks > {NCH}"

        fz = np.zeros((NCAP, TC), dtype=bf16)
        ranks_all = np.zeros((NCH, R), dtype=np.int64)
        rs = np.full((NCAP,), -1, dtype=np.int64)

        for k in range(len(chunk_starts)):
            p0, nr, span = chunk_starts[k], chunk_rows[k], chunk_spans[k]
            rows = order[p0:p0 + nr]
            segs = sidx[p0:p0 + nr]
            rank = np.zeros(nr, dtype=np.int64)
            rank[1:] = np.cumsum(segs[1:] != segs[:-1])
            base = k * R
            fz[base:base + nr, :256] = feats_f32[rows].astype(bf16)
            fz[base:base + R, 256] = 1.0
            rs[base:base + nr] = rows
            ranks_full = np.full(R, span, dtype=np.int64)  # pad rows -> pad slot
            ranks_full[:nr] = rank
            ranks_all[k] = ranks_full

        urz = ranks_all.reshape(NCH, T, 128).transpose(2, 0, 1).reshape(128, NCH * T)
        oh = (ranks_all[:, None, :] == np.arange(128)[None, :, None])
        urtz = oh.reshape(NBC, BC, 128, R).transpose(0, 2, 1, 3).reshape(NBC, 128, BC * R)

        # permute chunk-linear rows into the device block layout:
        # chunk k, sorted index i -> 2048*(k//BC) + (BC*T)*p + T*(k%BC) + t
        # with p = i % 128, t = i // 128
        kk = np.arange(NCH)[:, None]
        ii = np.arange(R)[None, :]
        pos = (R * BC) * (kk // BC) + (BC * T) * (ii % 128) + T * (kk % BC) + ii // 128
        pos_flat = pos.ravel()
        fz_b = np.zeros_like(fz)
        fz_b[pos_flat] = fz
        rs_b = np.full_like(rs, -1)
        rs_b[pos_flat] = rs
        fz, rs = fz_b, rs_b

        feats_list.append(fz)
        ur_list.append(np.ascontiguousarray(urz).astype(bf16))
        urt_list.append(np.ascontiguousarray(urtz).astype(fp8))
        rowsrc_list.append(rs)

    return feats_list, ur_list, urt_list, rowsrc_list


def kernel(intersect_rgb_feat, intersect_voxel_feat, miss_ray_intersect_idx,
           total_miss_sample_num, W):
    global LAST_EXEC_NS, LAST_RESULTS
    from concourse.bass_utils import run_bass_kernel_spmd

    rgb = np.asarray(intersect_rgb_feat, dtype=np.float32)
    vox = np.asarray(intersect_voxel_feat, dtype=np.float32)
    idx = np.asarray(miss_ray_intersect_idx).astype(np.int64)
    Wm = np.asarray(W, dtype=np.float32)
    assert rgb.shape == (N, 128) and vox.shape == (N, 128)
    assert int(total_miss_sample_num) == S

    feats_f32 = np.concatenate([rgb, vox], axis=1)
    feats_list, ur_list, urt_list, rowsrc_list = _prepare_shards(feats_f32, idx)

    wt_host = np.ascontiguousarray(Wm.T.reshape(2, 128, 256)).astype(bf16)

    nc = _build_graph()

    in_maps = []
    for c in range(NCORES):
        in_maps.append({
            "feats": feats_list[c],
            "ur": ur_list[c],
            "mskt_h": urt_list[c],
            "wt": wt_host,
        })

    trace = bool(os.environ.get("BASS_TRACE"))
    res = run_bass_kernel_spmd(nc, in_maps, core_ids=list(range(NCORES)),
                               trace=trace)
    LAST_EXEC_NS = res.exec_time_ns
    LAST_RESULTS = res

    out_full = np.zeros((N, D), dtype=np.float32)
    for c in range(NCORES):
        o = np.asarray(res.results[c]["out"]).astype(np.float32)
        rs = rowsrc_list[c]
        valid = rs >= 0
        out_full[rs[valid]] = o[valid]
    return out_full

